# revision 1
# baseline (speedup 1.0000x reference)
"""AUAvULoss kernel for Trainium2, data-parallel over N across 8 NeuronCores.

Two SPMD launches, all input streaming on the Pool SWDGE queue with
f32->bf16 dma-cast (the measured-fastest path, ~250 GB/s read):

  L1: streams probs (4 MB/core), computes per-sample entropy unc (f32),
      confidence conf, correctness corr (vs label_col), then the two
      contraction arrays
        D1 = corr*conf - u*E,  D2 = E*(1-2u),   u = tanh(unc),
        E  = corr*conf + (1-corr)*(1-conf)
      plus per-partition stats (min/max unc, sum D1, sum D2,
      sum u*E = P+Q, sum u*corr*conf).  PE transposes uncb/D1/D2 into
      sample-on-partition layout [128, 1000] which is written to DRAM.
  host: global min/max -> 21 thresholds; partial sums.
  L2: streams probs/y/w (12 MB/core) for CE/focal, reads back
      uncT/d12T, generates sign masks sgn_t = sign(th_t - unc) on the
      ACT engine (bias port carries th_t), and contracts on the PE:
      per 128-sample chunk  lhsT=d12T[:,c,:] [128,2], rhs=sgn[:,:,c]
      [128,21], accumulated into PSUM with 4-way column packing.
      sum(D*mask) = (sum(D*sgn) + sum(D))/2 recovers the LE sums.
  host: avu curve -> AUC -> losses.
"""

import sys
from contextlib import ExitStack

import numpy as np

for _p in ("/opt/trn_rl_repo",):
    if _p not in sys.path:
        sys.path.insert(0, _p)

import concourse.bacc as bacc
import concourse.bass as bass
import concourse.mybir as mybir
import concourse.tile as tile
from concourse.bass_utils import run_bass_kernel_spmd

f32 = mybir.dt.float32
bf16 = mybir.dt.bfloat16
AF = mybir.ActivationFunctionType
OP = mybir.AluOpType
AX = mybir.AxisListType

NCORES = 8
N, C = 1_000_000, 8
R = N // NCORES          # 125_000 rows per core
P = 125                  # SBUF partitions
W = R // P               # 1000 samples per partition
NCH = 4                  # input chunks
CW = W // NCH            # 250 samples/partition per chunk
CE_W = CW * C            # 2000 elements/partition per chunk
NTH = 21
NCHK = 1000              # transposed sample chunks (8 slabs x 125)
EPS = 1e-10
BETA = 1.0


def build_l1(label_col):
    nc = bacc.Bacc("TRN2", target_bir_lowering=False, debug=False,
                   enable_asserts=False, num_devices=NCORES)
    pr_d = nc.dram_tensor("probs", [R, C], f32, kind="ExternalInput").ap()
    uncT_d = nc.dram_tensor("uncT", [128, NCHK], bf16,
                            kind="ExternalOutput").ap()
    d12T_d = nc.dram_tensor("d12T", [128, 2 * NCHK], bf16,
                            kind="ExternalOutput").ap()
    st_d = nc.dram_tensor("st", [P, 8], f32, kind="ExternalOutput").ap()

    pr_r = pr_d.rearrange("(p w) c -> p (w c)", p=P)

    with tile.TileContext(nc) as tc, ExitStack() as ctx:
        io = ctx.enter_context(tc.tile_pool(name="io", bufs=2))
        sc = ctx.enter_context(tc.tile_pool(name="sc", bufs=2))
        ps = ctx.enter_context(tc.tile_pool(name="ps", bufs=1))
        psp = ctx.enter_context(tc.tile_pool(name="psp", bufs=1, space="PSUM"))

        unc_t = ps.tile([P, W], f32, tag="unc")
        confb = ps.tile([P, W], bf16, tag="confb")
        corrb = ps.tile([P, W], bf16, tag="corrb")
        # padded-to-1024 bf16 arrays for the PE transposes
        uncb = ps.tile([P, 1024], bf16, tag="uncb")
        d1b = ps.tile([P, 1024], bf16, tag="d1b")
        d2b = ps.tile([P, 1024], bf16, tag="d2b")
        nc.vector.memset(uncb[:, W:1024], 0.0)
        nc.gpsimd.memset(d1b[:, W:1024], 0.0)
        nc.gpsimd.memset(d2b[:, W:1024], 0.0)

        for k in range(NCH):
            sl = bass.ts(k, CE_W)
            pr = io.tile([P, CE_W], bf16, tag="pr")
            nc.gpsimd.dma_start(pr[:], pr_r[:, sl])

            lg = sc.tile([P, CE_W], bf16, tag="lg")
            nc.scalar.activation(lg[:], pr[:], AF.Ln)
            pl = sc.tile([P, CE_W], bf16, tag="pl")
            nc.gpsimd.tensor_tensor(pl[:], pr[:], lg[:], op=OP.mult)

            ksl = bass.ts(k, CW)
            pl3 = pl[:].rearrange("p (a c) -> p a c", c=C)
            pr3 = pr[:].rearrange("p (a c) -> p a c", c=C)
            nc.vector.tensor_reduce(unc_t[:, ksl], pl3, axis=AX.X,
                                    op=OP.add, negate=True)
            nc.vector.tensor_reduce(confb[:, ksl], pr3, axis=AX.X, op=OP.max)
            if label_col is not None:
                prL = pr3[:, :, label_col:label_col + 1]
                prL = prL.rearrange("p a c -> p (a c)")
                nc.vector.tensor_tensor(corrb[:, ksl], prL, confb[:, ksl],
                                        op=OP.is_ge)
            else:
                nc.vector.memset(corrb[:, ksl], 0.0)

        # epilogue on [P, W]
        ub = sc.tile([P, W], bf16, tag="ub")
        nc.scalar.activation(ub[:], unc_t[:], AF.Tanh)
        nc.scalar.activation(uncb[:, 0:W], unc_t[:], AF.Copy)
        cc = sc.tile([P, W], bf16, tag="cc")
        nc.vector.tensor_tensor(cc[:], confb[:], corrb[:], op=OP.mult)
        e1 = sc.tile([P, W], bf16, tag="e1")
        nc.gpsimd.tensor_tensor(e1[:], confb[:], corrb[:], op=OP.add)
        ta = sc.tile([P, W], bf16, tag="ta")
        nc.vector.tensor_tensor(ta[:], cc[:], e1[:], op=OP.subtract)
        tb = sc.tile([P, W], bf16, tag="tb")
        nc.gpsimd.tensor_tensor(tb[:], cc[:], ta[:], op=OP.add)
        ee = sc.tile([P, W], bf16, tag="ee")
        nc.vector.tensor_scalar_add(ee[:], tb[:], 1.0)
        t2 = sc.tile([P, W], bf16, tag="t2")
        nc.vector.tensor_tensor(t2[:], ub[:], ee[:], op=OP.mult)
        tc2 = sc.tile([P, W], bf16, tag="tc2")
        nc.gpsimd.tensor_tensor(tc2[:], t2[:], t2[:], op=OP.add)
        nc.vector.tensor_tensor(d2b[:, 0:W], ee[:], tc2[:], op=OP.subtract)
        nc.gpsimd.tensor_tensor(d1b[:, 0:W], cc[:], t2[:], op=OP.subtract)
        ucc = sc.tile([P, W], bf16, tag="ucc")
        nc.vector.tensor_tensor(ucc[:], ub[:], cc[:], op=OP.mult)

        st_t = ps.tile([P, 8], f32, tag="st")
        nc.vector.memset(st_t[:, 6:8], 0.0)
        nc.vector.tensor_reduce(st_t[:, 0:1], unc_t[:], axis=AX.X, op=OP.min)
        nc.vector.tensor_reduce(st_t[:, 1:2], unc_t[:], axis=AX.X, op=OP.max)
        nc.vector.tensor_reduce(st_t[:, 2:3], d1b[:, 0:W], axis=AX.X,
                                op=OP.add)
        nc.vector.tensor_reduce(st_t[:, 3:4], d2b[:, 0:W], axis=AX.X,
                                op=OP.add)
        nc.vector.tensor_reduce(st_t[:, 4:5], t2[:], axis=AX.X, op=OP.add)
        nc.vector.tensor_reduce(st_t[:, 5:6], ucc[:], axis=AX.X, op=OP.add)

        # identity for PE transposes
        ones_t = ps.tile([P, P], bf16, tag="ones")
        nc.vector.memset(ones_t[:], 1.0)
        ident = ps.tile([P, P], bf16, tag="ident")
        nc.gpsimd.affine_select(ident[:], ones_t[:], [[-1, P]],
                                OP.is_equal, 0.0, base=0, channel_multiplier=1)

        psU = psp.tile([128, 8, 128], bf16, tag="psU")
        psD1 = psp.tile([128, 8, 128], bf16, tag="psD1")
        psD2 = psp.tile([128, 8, 128], bf16, tag="psD2")
        for s in range(8):
            ssl = bass.ts(s, 128)
            nc.tensor.transpose(psU[:, s, 0:P], uncb[:, ssl], ident[:])
            nc.tensor.transpose(psD1[:, s, 0:P], d1b[:, ssl], ident[:])
            nc.tensor.transpose(psD2[:, s, 0:P], d2b[:, ssl], ident[:])

        uncT_t = ps.tile([128, NCHK], bf16, tag="uncT")
        uv = uncT_t[:].rearrange("p (s c) -> p s c", c=P)
        nc.vector.tensor_copy(uv, psU[:, :, 0:P])
        d12T_t = ps.tile([128, NCHK, 2], bf16, tag="d12T")
        d1v = d12T_t[:, :, 0].rearrange("p (s c) -> p s c", c=P)
        nc.vector.tensor_copy(d1v, psD1[:, :, 0:P])
        d2v = d12T_t[:, :, 1].rearrange("p (s c) -> p s c", c=P)
        nc.scalar.copy(d2v, psD2[:, :, 0:P])

        nc.gpsimd.dma_start(uncT_d[:, :], uncT_t[:])
        nc.gpsimd.dma_start(
            d12T_d[:, :].rearrange("p (c q) -> p c q", q=2), d12T_t[:])
        nc.gpsimd.dma_start(st_d[:, :], st_t[:])

    nc.compile()
    return nc


def build_l2():
    nc = bacc.Bacc("TRN2", target_bir_lowering=False, debug=False,
                   enable_asserts=False, num_devices=NCORES)
    pr_d = nc.dram_tensor("probs", [R, C], f32, kind="ExternalInput").ap()
    y_d = nc.dram_tensor("y", [R, C], f32, kind="ExternalInput").ap()
    w_d = nc.dram_tensor("w", [R, C], f32, kind="ExternalInput").ap()
    uncT_d = nc.dram_tensor("uncT", [128, NCHK], bf16,
                            kind="ExternalInput").ap()
    d12T_d = nc.dram_tensor("d12T", [128, 2 * NCHK], bf16,
                            kind="ExternalInput").ap()
    th_d = nc.dram_tensor("th", [128, NTH], f32, kind="ExternalInput").ap()
    S_d = nc.dram_tensor("S", [128, NTH], f32, kind="ExternalOutput").ap()
    ce_d = nc.dram_tensor("ce", [P, NCH], f32, kind="ExternalOutput").ap()
    fo_d = nc.dram_tensor("fo", [P, NCH], f32, kind="ExternalOutput").ap()

    pr_r = pr_d.rearrange("(p w) c -> p (w c)", p=P)
    y_r = y_d.rearrange("(p w) c -> p (w c)", p=P)
    w_r = w_d.rearrange("(p w) c -> p (w c)", p=P)

    with tile.TileContext(nc) as tc, ExitStack() as ctx:
        io = ctx.enter_context(tc.tile_pool(name="io", bufs=2))
        sc = ctx.enter_context(tc.tile_pool(name="sc", bufs=2))
        ps = ctx.enter_context(tc.tile_pool(name="ps", bufs=1))
        psp = ctx.enter_context(tc.tile_pool(name="psp", bufs=1, space="PSUM"))

        uncT_t = ps.tile([128, NCHK], bf16, tag="uncT")
        nc.gpsimd.dma_start(uncT_t[:], uncT_d[:, :])
        d12T_t = ps.tile([128, NCHK, 2], bf16, tag="d12T")
        nc.gpsimd.dma_start(
            d12T_t[:], d12T_d[:, :].rearrange("p (c q) -> p c q", q=2))
        th_t = ps.tile([128, NTH], f32, tag="th")
        nc.gpsimd.dma_start(th_t[:], th_d[:, :])

        ce_acc = ps.tile([P, NCH], f32, tag="ceacc")
        fo_acc = ps.tile([P, NCH], f32, tag="foacc")
        ones_t = ps.tile([P, 1], bf16, tag="ones")
        nc.vector.memset(ones_t[:], 1.0)

        # CE / focal path while everything streams
        for k in range(NCH):
            sl = bass.ts(k, CE_W)
            pr = io.tile([P, CE_W], bf16, tag="pr")
            nc.gpsimd.dma_start(pr[:], pr_r[:, sl])
            yy = io.tile([P, CE_W], bf16, tag="yy")
            nc.gpsimd.dma_start(yy[:], y_r[:, sl])
            ww = io.tile([P, CE_W], bf16, tag="ww")
            nc.gpsimd.dma_start(ww[:], w_r[:, sl])

            lg = sc.tile([P, CE_W], bf16, tag="lg")
            nc.scalar.activation(lg[:], pr[:], AF.Ln)
            t1 = sc.tile([P, CE_W], bf16, tag="t1")
            nc.gpsimd.tensor_tensor(t1[:], yy[:], lg[:], op=OP.mult)
            junka = sc.tile([P, CE_W], bf16, tag="junka")
            nc.scalar.activation(junka[:], t1[:], AF.Copy,
                                 accum_out=ce_acc[:, k:k + 1])
            junkb = sc.tile([P, CE_W], bf16, tag="junkb")
            nc.vector.scalar_tensor_tensor(
                out=junkb[:], in0=t1[:], scalar=ones_t[:, 0:1], in1=ww[:],
                op0=OP.mult, op1=OP.mult, accum_out=fo_acc[:, k:k + 1])

        # sign masks on ACT: sgn[:, t, c] = sign(th_t - unc_c)
        sgn = ps.tile([128, NTH, NCHK], bf16, tag="sgn")
        GC = NCHK // 4
        for g in range(4):
            gsl = bass.ts(g, GC)
            for t in range(NTH):
                nc.scalar.activation(sgn[:, t, gsl], uncT_t[:, gsl],
                                     AF.Sign, bias=th_t[:, t:t + 1],
                                     scale=-1.0)

        psum = psp.tile([128, NTH], f32, tag="acc")
        nc.vector.memset(psum[:], 0.0)
        for c in range(NCHK):
            i = c % 4
            nc.tensor.matmul(psum[32 * i:32 * i + 2, :],
                             d12T_t[:, c, :], sgn[:, :, c],
                             start=False, stop=(c >= NCHK - 4),
                             skip_group_check=True,
                             tile_position=(0, 32 * i))

        S_t = ps.tile([128, NTH], f32, tag="S")
        nc.vector.tensor_copy(S_t[:], psum[:])
        nc.gpsimd.dma_start(S_d[:, :], S_t[:])
        nc.gpsimd.dma_start(ce_d[:, :], ce_acc[:])
        nc.gpsimd.dma_start(fo_d[:, :], fo_acc[:])

    nc.compile()
    return nc


_cache = {}


def _get_l1(label_col):
    key = ("l1", label_col)
    if key not in _cache:
        _cache[key] = build_l1(label_col)
    return _cache[key]


def _get_l2():
    if "l2" not in _cache:
        _cache["l2"] = build_l2()
    return _cache["l2"]


def kernel(probs, y, weights, _results=None, _trace=False):
    probs = np.ascontiguousarray(probs, dtype=np.float32)
    y = np.ascontiguousarray(y, dtype=np.float32)
    weights = np.ascontiguousarray(weights, dtype=np.float32)

    flat_label = int(np.argmax(y))
    label_col = flat_label if flat_label < C else None

    nc1 = _get_l1(label_col)
    in1 = [{"probs": probs[i * R:(i + 1) * R]} for i in range(NCORES)]
    tr1 = {"trace": True, "tmpdir": "/tmp/trace_k1"} if _trace else {}
    if _trace:
        import os as _os
        import shutil as _sh
        for d in ("/tmp/trace_k1", "/tmp/trace_k2"):
            _sh.rmtree(d, ignore_errors=True)
            _os.makedirs(d, exist_ok=True)
    r1 = run_bass_kernel_spmd(nc1, in1, core_ids=list(range(NCORES)), **tr1)
    outs1 = r1.results

    st = np.stack([o["st"] for o in outs1])          # [cores, P, 8]
    umin = float(st[:, :, 0].min())
    umax = float(st[:, :, 1].max())
    SD1 = float(st[:, :, 2].sum(dtype=np.float64))
    SD2 = float(st[:, :, 3].sum(dtype=np.float64))
    PQ_tot = float(st[:, :, 4].sum(dtype=np.float64))
    uCC_tot = float(st[:, :, 5].sum(dtype=np.float64))
    Q_tot = PQ_tot - uCC_tot

    th01 = np.linspace(0.0, 1.0, NTH).astype(np.float32)
    unc_th = (np.float32(umin) + th01 *
              (np.float32(umax) - np.float32(umin))).astype(np.float32)
    th_b = np.broadcast_to(unc_th, (128, NTH)).copy()

    nc2 = _get_l2()
    in2 = [{"probs": probs[i * R:(i + 1) * R],
            "y": y[i * R:(i + 1) * R],
            "w": weights[i * R:(i + 1) * R],
            "uncT": outs1[i]["uncT"],
            "d12T": outs1[i]["d12T"],
            "th": th_b} for i in range(NCORES)]
    tr2 = {"trace": True, "tmpdir": "/tmp/trace_k2"} if _trace else {}
    r2 = run_bass_kernel_spmd(nc2, in2, core_ids=list(range(NCORES)), **tr2)
    outs2 = r2.results

    ce_sum = sum(float(o["ce"].sum(dtype=np.float64)) for o in outs2)
    fo_sum = sum(float(o["fo"].sum(dtype=np.float64)) for o in outs2)
    CE_loss = -ce_sum / N
    focal_loss = -fo_sum / N

    Sp = np.zeros((2, NTH), dtype=np.float64)
    for o in outs2:
        a = o["S"].astype(np.float64)
        for i in range(4):
            Sp += a[32 * i:32 * i + 2, :]
    S1 = (Sp[0] + SD1) / 2.0
    S2 = (Sp[1] + SD2) / 2.0

    num = Q_tot + S1
    den = PQ_tot + S2
    avu = num / (den + EPS)
    dx = np.diff(th01.astype(np.float64))
    auc_avu = float(np.sum(0.5 * (avu[1:] + avu[:-1]) * dx))
    avu_loss = -BETA * np.log(auc_avu + EPS) + focal_loss

    if _results is not None:
        _results.update(r1=r1, r2=r2, umin=umin, umax=umax, avu=avu,
                        auc=auc_avu)
    return (np.float32(avu_loss), np.float32(CE_loss))



# revision 4
# speedup vs baseline: 1.2057x; 1.2057x over previous
"""AUAvULoss kernel for Trainium2, data-parallel over N across 8 NeuronCores.

Single SPMD launch. Per core (125K rows):
  - probs/y/w stream on the Pool SWDGE queue with f32->bf16 dma-cast,
    8 chunks of [125, 1000] each, probs first.
  - probs phase: lg = Ln(probs) on ACT (kept in SBUF for CE/focal),
    pl = p*lg (DVE), per-sample entropy uncb / confidence confb via
    DVE C-reduces, correctness corrb vs the (faithful-bug) scalar label.
  - epilogue: u = tanh(unc), E = cc + (1-corr)(1-conf),
    D1 = cc - u*E, D2 = E*(1-2u), plus per-partition stats
    (max(-u), max(u), sum u*E, sum u*cc, sum D1, sum D2).
  - cross-core: per-core (-umin, umax) is partition-reduced via a PE
    transpose, AllReduce(max) over the 8 cores gives global min/max,
    thresholds th_t = umin + t/20*(umax-umin) are built on-device.
  - y/w phase (overlaps all of the below): ce = sum y*lg and
    fo = sum (y*w)*lg per chunk via DVE scalar_tensor_tensor accums.
  - PE transposes uncb/D1/D2 into sample-on-partition layout; ACT
    generates sgn_t = sign(th_t - unc); PE contracts
    lhsT=d12T[:,c,:] [128,2] x rhs=sgn[:,:,c] [128,21] into PSUM with
    4-way column packing.  sum(D*mask) = (sum(D*sgn)+sum(D))/2.
  - host: reduce the per-core scalars, avu curve -> AUC -> losses.
"""

import sys
from contextlib import ExitStack

import numpy as np

for _p in ("/opt/trn_rl_repo",):
    if _p not in sys.path:
        sys.path.insert(0, _p)

import concourse.bacc as bacc
import concourse.bass as bass
import concourse.mybir as mybir
import concourse.tile as tile
from concourse.bass_utils import run_bass_kernel_spmd

f32 = mybir.dt.float32
bf16 = mybir.dt.bfloat16
AF = mybir.ActivationFunctionType
OP = mybir.AluOpType
AX = mybir.AxisListType

NCORES = 8
N, C = 1_000_000, 8
R = N // NCORES          # 125_000 rows per core
P = 125                  # SBUF partitions
W = R // P               # 1000 samples per partition
FW = W * C               # 8000 elements per partition
NCH = 8                  # input chunks per tensor
CW = FW // NCH           # 1000 elements/partition per chunk
SW = W // NCH            # 125 samples/partition per chunk
NTH = 21
NCHK = 1000              # transposed sample chunks (8 slabs x 125)
EPS = 1e-10
BETA = 1.0


def build(label_col):
    nc = bacc.Bacc("TRN2", target_bir_lowering=False, debug=False,
                   enable_asserts=False, num_devices=NCORES)
    pr_d = nc.dram_tensor("probs", [R, C], f32, kind="ExternalInput").ap()
    y_d = nc.dram_tensor("y", [R, C], f32, kind="ExternalInput").ap()
    w_d = nc.dram_tensor("w", [R, C], f32, kind="ExternalInput").ap()
    c21_d = nc.dram_tensor("c21", [128, NTH], f32, kind="ExternalInput").ap()
    S_d = nc.dram_tensor("S", [128, NTH], f32, kind="ExternalOutput").ap()
    st_d = nc.dram_tensor("st", [P, 24], f32, kind="ExternalOutput").ap()

    pr_r = pr_d.rearrange("(p w) c -> p (w c)", p=P)
    y_r = y_d.rearrange("(p w) c -> p (w c)", p=P)
    w_r = w_d.rearrange("(p w) c -> p (w c)", p=P)

    with tile.TileContext(nc) as tc, ExitStack() as ctx:
        io = ctx.enter_context(tc.tile_pool(name="io", bufs=3))
        sc = ctx.enter_context(tc.tile_pool(name="sc", bufs=4))
        ps = ctx.enter_context(tc.tile_pool(name="ps", bufs=1))
        psp = ctx.enter_context(tc.tile_pool(name="psp", bufs=1, space="PSUM"))
        dram = ctx.enter_context(tc.tile_pool(name="dram", bufs=2,
                                              space="DRAM"))

        c21_t = ps.tile([128, NTH], f32, tag="c21")
        nc.gpsimd.dma_start(c21_t[:], c21_d[:, :])

        lg_full = ps.tile([P, FW], bf16, tag="lg")
        confb = ps.tile([P, W], bf16, tag="confb")
        corrb = ps.tile([P, W], bf16, tag="corrb")
        # padded-to-1024 bf16 arrays for the PE transposes
        uncb = ps.tile([P, 1024], bf16, tag="uncb")
        d1b = ps.tile([P, 1024], bf16, tag="d1b")
        d2b = ps.tile([P, 1024], bf16, tag="d2b")
        nc.vector.memset(uncb[:, W:1024], 0.0)
        nc.gpsimd.memset(d1b[:, W:1024], 0.0)
        nc.gpsimd.memset(d2b[:, W:1024], 0.0)

        st_t = ps.tile([P, 24], f32, tag="st")
        nc.vector.memset(st_t[:], 0.0)

        # ---------------- phase A: probs ----------------
        for k in range(NCH):
            sl = bass.ts(k, CW)
            ssl = bass.ts(k, SW)
            pr = io.tile([P, CW], bf16, tag="pr")
            nc.gpsimd.dma_start(pr[:], pr_r[:, sl])
            nc.scalar.activation(lg_full[:, sl], pr[:], AF.Ln)
            pl = sc.tile([P, CW], bf16, tag="pl")
            nc.vector.tensor_tensor(pl[:], pr[:], lg_full[:, sl], op=OP.mult)
            pl3 = pl[:].rearrange("p (a c) -> p a c", c=C)
            pr3 = pr[:].rearrange("p (a c) -> p a c", c=C)
            with nc.allow_low_precision(reason="8-elem entropy reduce"):
                nc.vector.tensor_reduce(uncb[:, ssl], pl3, axis=AX.X,
                                        op=OP.add, negate=True)
            nc.vector.tensor_reduce(confb[:, ssl], pr3, axis=AX.X, op=OP.max)
            if label_col is not None:
                prL = pr3[:, :, label_col:label_col + 1]
                prL = prL.rearrange("p a c -> p (a c)")
                nc.vector.tensor_tensor(corrb[:, ssl], prL, confb[:, ssl],
                                        op=OP.is_ge)
            else:
                nc.vector.memset(corrb[:, ssl], 0.0)

        # ---------------- epilogue on [P, W] ----------------
        ub = sc.tile([P, W], bf16, tag="ub")
        nc.scalar.activation(ub[:], uncb[:, 0:W], AF.Tanh)
        cc = sc.tile([P, W], bf16, tag="cc")
        nc.vector.tensor_tensor(cc[:], confb[:], corrb[:], op=OP.mult)
        e1 = sc.tile([P, W], bf16, tag="e1")
        nc.gpsimd.tensor_tensor(e1[:], confb[:], corrb[:], op=OP.add)
        ta = sc.tile([P, W], bf16, tag="ta")
        nc.vector.tensor_tensor(ta[:], cc[:], e1[:], op=OP.subtract)
        tb = sc.tile([P, W], bf16, tag="tb")
        nc.gpsimd.tensor_tensor(tb[:], cc[:], ta[:], op=OP.add)
        ee = sc.tile([P, W], bf16, tag="ee")
        nc.vector.tensor_scalar_add(ee[:], tb[:], 1.0)
        t2 = sc.tile([P, W], bf16, tag="t2")
        nc.vector.tensor_tensor(t2[:], ub[:], ee[:], op=OP.mult)
        tc2 = sc.tile([P, W], bf16, tag="tc2")
        nc.gpsimd.tensor_tensor(tc2[:], t2[:], t2[:], op=OP.add)
        nc.vector.tensor_tensor(d2b[:, 0:W], ee[:], tc2[:], op=OP.subtract)
        nc.gpsimd.tensor_tensor(d1b[:, 0:W], cc[:], t2[:], op=OP.subtract)
        ucc = sc.tile([P, W], bf16, tag="ucc")
        nc.vector.tensor_tensor(ucc[:], ub[:], cc[:], op=OP.mult)

        # stats: st cols 16=PQ, 17=uCC, 18=SD1, 19=SD2
        nc.vector.tensor_reduce(st_t[:, 16:17], t2[:], axis=AX.X, op=OP.add)
        nc.vector.tensor_reduce(st_t[:, 17:18], ucc[:], axis=AX.X, op=OP.add)
        nc.vector.tensor_reduce(st_t[:, 18:19], d1b[:, 0:W], axis=AX.X,
                                op=OP.add)
        nc.vector.tensor_reduce(st_t[:, 19:20], d2b[:, 0:W], axis=AX.X,
                                op=OP.add)

        # ---------------- global min/max via AllReduce ----------------
        # mm2 bf16 [P, 2] = (max(-u), max(u)); bf16 min/max is exact on
        # the bf16 unc values.
        mm2 = ps.tile([P, 2], bf16, tag="mm2")
        mmn = ps.tile([P, 1], bf16, tag="mmn")
        with nc.allow_low_precision(reason="bf16 max is exact"):
            nc.vector.tensor_reduce(mmn[:], uncb[:, 0:W], axis=AX.X,
                                    op=OP.min)
            nc.vector.tensor_reduce(mm2[:, 1:2], uncb[:, 0:W], axis=AX.X,
                                    op=OP.max)
        nc.scalar.activation(mm2[:, 0:1], mmn[:], AF.Copy, scale=-1.0)

        # identity for PE transposes
        ones_t = ps.tile([P, P], bf16, tag="ones")
        nc.vector.memset(ones_t[:], 1.0)
        ident = ps.tile([P, P], bf16, tag="ident")
        nc.gpsimd.affine_select(ident[:], ones_t[:], [[-1, P]],
                                OP.is_equal, 0.0, base=0, channel_multiplier=1)

        # partition-reduce (-umin, umax) via PE transpose
        psMM = psp.tile([2, P], bf16, tag="psMM")
        nc.tensor.transpose(psMM[:, 0:P], mm2[:], ident[:])
        mmT = ps.tile([2, P], bf16, tag="mmT")
        nc.scalar.copy(mmT[:], psMM[:, 0:P])
        ccv = ps.tile([2, 1], f32, tag="ccv")
        nc.vector.tensor_reduce(ccv[:], mmT[:], axis=AX.X, op=OP.max)

        cin = dram.tile([2, 1], f32)
        cout = dram.tile([2, 1], f32)
        nc.gpsimd.dma_start(cin[:], ccv[:])
        nc.gpsimd.collective_compute(
            "AllReduce", OP.max, replica_groups=[list(range(NCORES))],
            ins=[cin.opt()], outs=[cout.opt()])
        ccb = ps.tile([128, 2], f32, tag="ccb")
        nc.gpsimd.dma_start(
            ccb[:], cout[:].rearrange("a b -> b a").to_broadcast([128, 2]))

        # th[p, t] = umin + c21[t]*(umax - umin)
        #          = c21[t]*(umax + negumin) - negumin
        delta = ps.tile([128, 1], f32, tag="delta")
        nc.vector.tensor_tensor(delta[:], ccb[:, 0:1], ccb[:, 1:2], op=OP.add)
        th1 = ps.tile([128, NTH], f32, tag="th1")
        nc.vector.tensor_tensor(th1[:], c21_t[:],
                                delta[:].to_broadcast([128, NTH]), op=OP.mult)
        th_t = ps.tile([128, NTH], f32, tag="th")
        nc.vector.tensor_tensor(th_t[:], th1[:],
                                ccb[:, 0:1].to_broadcast([128, NTH]),
                                op=OP.subtract)

        # ---------------- phase B: y / w (CE, focal) ----------------
        ce8 = ps.tile([P, NCH], f32, tag="ce8")
        fo8 = ps.tile([P, NCH], f32, tag="fo8")
        for k in range(NCH):
            sl = bass.ts(k, CW)
            yy = io.tile([P, CW], bf16, tag="yy")
            nc.gpsimd.dma_start(yy[:], y_r[:, sl])
            ww = io.tile([P, CW], bf16, tag="ww")
            nc.gpsimd.dma_start(ww[:], w_r[:, sl])

            junka = sc.tile([P, CW], bf16, tag="junka")
            nc.vector.scalar_tensor_tensor(
                out=junka[:], in0=yy[:], scalar=1.0, in1=lg_full[:, sl],
                op0=OP.mult, op1=OP.mult, accum_out=ce8[:, k:k + 1])
            t1 = sc.tile([P, CW], bf16, tag="t1")
            nc.vector.tensor_tensor(t1[:], yy[:], ww[:], op=OP.mult)
            junkb = sc.tile([P, CW], bf16, tag="junkb")
            nc.vector.scalar_tensor_tensor(
                out=junkb[:], in0=t1[:], scalar=1.0, in1=lg_full[:, sl],
                op0=OP.mult, op1=OP.mult, accum_out=fo8[:, k:k + 1])

        # ---------------- PE transposes ----------------
        psU = psp.tile([128, 8, 128], bf16, tag="psU")
        psD1 = psp.tile([128, 8, 128], bf16, tag="psD1")
        psD2 = psp.tile([128, 8, 128], bf16, tag="psD2")
        for s in range(8):
            ssl = bass.ts(s, 128)
            nc.tensor.transpose(psU[:, s, 0:P], uncb[:, ssl], ident[:])
            nc.tensor.transpose(psD1[:, s, 0:P], d1b[:, ssl], ident[:])
            nc.tensor.transpose(psD2[:, s, 0:P], d2b[:, ssl], ident[:])

        uncT_t = ps.tile([128, NCHK], bf16, tag="uncT")
        uv = uncT_t[:].rearrange("p (s c) -> p s c", c=P)
        nc.vector.tensor_copy(uv, psU[:, :, 0:P])
        d12T_t = ps.tile([128, NCHK, 2], bf16, tag="d12T")
        d1v = d12T_t[:, :, 0].rearrange("p (s c) -> p s c", c=P)
        nc.vector.tensor_copy(d1v, psD1[:, :, 0:P])
        d2v = d12T_t[:, :, 1].rearrange("p (s c) -> p s c", c=P)
        nc.scalar.copy(d2v, psD2[:, :, 0:P])

        # ---------------- sgn masks + PE contraction ----------------
        sgn = ps.tile([128, NTH, NCHK], bf16, tag="sgn")
        GC = NCHK // 4
        for g in range(4):
            gsl = bass.ts(g, GC)
            for t in range(NTH):
                nc.scalar.activation(sgn[:, t, gsl], uncT_t[:, gsl],
                                     AF.Sign, bias=th_t[:, t:t + 1],
                                     scale=-1.0)

        psum = psp.tile([128, NTH], f32, tag="acc")
        nc.vector.memset(psum[:], 0.0)
        for c in range(NCHK):
            i = c % 4
            nc.tensor.matmul(psum[32 * i:32 * i + 2, :],
                             d12T_t[:, c, :], sgn[:, :, c],
                             start=False, stop=(c >= NCHK - 4),
                             skip_group_check=True,
                             tile_position=(0, 32 * i))

        S_t = ps.tile([128, NTH], f32, tag="S")
        nc.vector.tensor_copy(S_t[:], psum[:])
        nc.gpsimd.dma_start(S_d[:, :], S_t[:])
        # debug: cols 20:22 = allreduced (negumin, umax); 22:23 = own mm
        nc.vector.tensor_copy(st_t[:, 20:22], ccb[0:P, 0:2])
        nc.scalar.copy(st_t[:, 22:23], mm2[:, 0:1])
        nc.scalar.copy(st_t[:, 23:24], mm2[:, 1:2])
        # ce/fo chunk accums into st cols 0:8 / 8:16
        nc.vector.tensor_copy(st_t[:, 0:NCH], ce8[:])
        nc.vector.tensor_copy(st_t[:, 8:8 + NCH], fo8[:])
        nc.gpsimd.dma_start(st_d[:, :], st_t[:])

    nc.compile()
    return nc


_cache = {}


def _get(label_col):
    key = ("l1", label_col)
    if key not in _cache:
        _cache[key] = build(label_col)
    return _cache[key]


def kernel(probs, y, weights, _results=None, _trace=False):
    probs = np.ascontiguousarray(probs, dtype=np.float32)
    y = np.ascontiguousarray(y, dtype=np.float32)
    weights = np.ascontiguousarray(weights, dtype=np.float32)

    flat_label = int(np.argmax(y))
    label_col = flat_label if flat_label < C else None

    nc1 = _get(label_col)
    c21 = np.broadcast_to(
        np.linspace(0.0, 1.0, NTH, dtype=np.float32), (128, NTH)).copy()
    in1 = [{"probs": probs[i * R:(i + 1) * R],
            "y": y[i * R:(i + 1) * R],
            "w": weights[i * R:(i + 1) * R],
            "c21": c21} for i in range(NCORES)]
    tr1 = {"trace": True, "tmpdir": "/tmp/trace_k1"} if _trace else {}
    if _trace:
        import os as _os
        import shutil as _sh
        _sh.rmtree("/tmp/trace_k1", ignore_errors=True)
        _os.makedirs("/tmp/trace_k1", exist_ok=True)
    r1 = run_bass_kernel_spmd(nc1, in1, core_ids=list(range(NCORES)), **tr1)
    outs = r1.results

    st = np.stack([o["st"] for o in outs])            # [cores, P, 24]
    ce_sum = float(st[:, :, 0:8].sum(dtype=np.float64))
    fo_sum = float(st[:, :, 8:16].sum(dtype=np.float64))
    PQ_tot = float(st[:, :, 16].sum(dtype=np.float64))
    uCC_tot = float(st[:, :, 17].sum(dtype=np.float64))
    SD1 = float(st[:, :, 18].sum(dtype=np.float64))
    SD2 = float(st[:, :, 19].sum(dtype=np.float64))
    Q_tot = PQ_tot - uCC_tot

    CE_loss = -ce_sum / N
    focal_loss = -fo_sum / N

    Sp = np.zeros((2, NTH), dtype=np.float64)
    for o in outs:
        a = o["S"].astype(np.float64)
        for i in range(4):
            Sp += a[32 * i:32 * i + 2, :]
    S1 = (Sp[0] + SD1) / 2.0
    S2 = (Sp[1] + SD2) / 2.0

    num = Q_tot + S1
    den = PQ_tot + S2
    avu = num / (den + EPS)
    th01 = np.linspace(0.0, 1.0, NTH)
    dx = np.diff(th01)
    auc_avu = float(np.sum(0.5 * (avu[1:] + avu[:-1]) * dx))
    avu_loss = -BETA * np.log(auc_avu + EPS) + focal_loss

    if _results is not None:
        _results.update(r1=r1, avu=avu, auc=auc_avu)
    return (np.float32(avu_loss), np.float32(CE_loss))
